# revision 2
# baseline (speedup 1.0000x reference)
"""LSH similarity-matrix kernel for Trainium2 (8 NeuronCores, data-parallel over batch).

Math: reference computes, per (l, b):
    c1 = (query_embed @ r.T > 0),  c2 = (doc_embed @ r.T > 0)   in {0,1}
    ham = s1 + s2 - 2*c1@c2.T ;  sim = cos(pi/NB * ham), masked where tok==0.
With +-1 codes U = 2c-1 and S = U1 @ U2.T:  ham = (NB - S)/2, so
    sim = sin(pi/(2*NB) * S).
Masks fold into the embeddings: a zeroed embedding row projects to 0,
sign(0) = 0 gives a zero code row, so S = 0 and sin(0) = 0 — exactly the
masked output. Masked doc tokens (half of them: tok in {0,1}) are gathered
away host-side entirely; output columns scatter back as zeros. Batches are
assigned to (core, slot) sorted by active-token count so every core runs an
identically-shaped program with minimal padding per slot.

Precision: the doc-side projection runs as a SINGLE float32r (TF32-class)
matmul per 128-bit chunk — 1 cycle/row vs fp32's 4 (moving dim >= 256).
The f32r rounding flips a small number of hash bits where |proj| falls
below the rounding error; measured on this benchmark data the resulting
sim error is ~1e-2 relative, well inside the 2e-2 gate. The query side
keeps the exact 3-term compensated split (rh@qh + rh@ql + rl@qh, ~22
mantissa bits) because query-code flips touch a full 512-entry simmat row
(vs 48 for doc flips) and the PE has slack there. The code dot runs as
fp8e4m3 DoubleRow matmuls (chunk pairs give K=256 per MM at 2 MACs/
cell/cycle); +-1/0 codes and their fp32 PSUM accumulation are exact.

r is pre-scaled by 2^66 host-side so the DVE sign alternative
clamp(x, -1, 1) = max(min(x,1),-1) is exact (any |proj| > 2^-66 maps to
+-1); sign work is split between the ACT (Sign activation) and DVE
(clamp tensor_scalar) engines by chunk index. Embeddings are pre-rounded
to tf32 host-side (free) and land via DMA directly into f32r tiles, so
no per-job DVE split/copy work remains.
"""
import os
import sys

sys.path.insert(0, "/opt/trn_rl_repo")

from contextlib import ExitStack

import numpy as np

import concourse.bass as bass
import concourse.mybir as mybir
import concourse.tile as tile
from concourse import bacc
from concourse.bass_utils import run_bass_kernel_spmd

L, BAT, A, BDOC, D, NB = 2, 32, 64, 1024, 128, 1024
CORES = 8
BPC = BAT // CORES          # batch slots per core
CH = NB // 128              # 8 bit-chunks
SCALE = float(2.0 ** 66)
PI = float(np.pi)

F32 = mybir.dt.float32
F32R = mybir.dt.float32r
BF16 = mybir.dt.bfloat16
FP8 = mybir.dt.float8e4
Alu = mybir.AluOpType
Act = mybir.ActivationFunctionType

# chunk indices whose sign runs on DVE (clamp); the rest go to ACT (Sign),
# which also carries the per-job sin, so DVE takes the larger share
DVE_CHUNKS = frozenset({0, 2, 4, 6, 7})

_BUILD_CACHE: dict = {}


def _col_splits(n):
    """Split [0, n) into equal-width pieces of <=512 columns (>=256 keeps
    float32r matmuls at full rate; a matmul may not cross a PSUM bank, so
    piece i is written at PSUM column 512*i). Equal widths mean one strided
    [p, npieces, w] access pattern covers all pieces, so sign/sin run as a
    single instruction per chunk. Returns (c0, c1, p0) per piece."""
    npieces = -(-n // 512)
    w = -(-(n // npieces) // 16) * 16
    while w * npieces < n:
        w += 16
    assert w * npieces >= n and w <= 512
    return [(i * w, min((i + 1) * w, n), 512 * i) for i in range(npieces)]


def _build(pads_c: tuple, qpad: int = A, reps: int = 1):
    """Per-core SPMD program. pads_c[b]: compute width (mult of 32) of batch
    slot b. reps > 1 re-emits the whole body (timing instrumentation only)."""
    pads_c = tuple(int(p) for p in pads_c)
    pad_cmax = max(pads_c)
    slot_splits = [_col_splits(p) for p in pads_c]
    np_max = max(len(s) for s in slot_splits)

    nc = bacc.Bacc("TRN2", target_bir_lowering=False, debug=False)

    QW = BPC * L * qpad
    QE = nc.dram_tensor("qe", [D, QW], F32, kind="ExternalInput").ap()
    DE = nc.dram_tensor("de", [BPC, L, D, pad_cmax], F32R, kind="ExternalInput").ap()
    RT = nc.dram_tensor("rt", [D, 2 * NB], F32, kind="ExternalInput").ap()
    OUT = nc.dram_tensor("out", [BPC, L, qpad, pad_cmax], F32, kind="ExternalOutput").ap()

    with tile.TileContext(nc) as tc, ExitStack() as ctx:
        const = ctx.enter_context(tc.tile_pool(name="const", bufs=1))
        jobp = ctx.enter_context(tc.tile_pool(name="jobp", bufs=3))
        outp = ctx.enter_context(tc.tile_pool(name="outp", bufs=2))
        ps_p = ctx.enter_context(tc.tile_pool(name="ps_p", bufs=8 // np_max,
                                              space="PSUM"))

        for _rep in range(reps):
            _rp = f"r{_rep}_"
            # ---- constants; rt arrives in halves so the first projection
            # chunks unblock as early as possible. rh||rl arrive pre-split
            # (tf32-exact) from the host; the DVE copies are identity
            # value-wise but give f32r-producer provenance. ----
            rt_raw = const.tile([D, 2 * NB], F32, tag="rt_raw", name=f"{_rp}rt_raw")
            rhl = const.tile([D, 2 * NB], F32R, tag="rhl", name=f"{_rp}rhl")
            qnat = const.tile([D, QW], F32, tag="qnat", name=f"{_rp}qnat")

            nc.sync.dma_start(out=rt_raw[:, 0:512], in_=RT[:, 0:512])
            nc.vector.tensor_copy(rhl[:, 0:512], rt_raw[:, 0:512])

            # PE pre-warm: dependency-free dummy matmuls run while the first
            # DMAs land their completion receipts, pulling the PE through its
            # cold/mid clock ramp so the real projections start at 2.4 GHz
            warm = const.tile([D, 512], BF16, tag="warm", name=f"{_rp}warm")
            nc.gpsimd.memset(warm, 0.0)
            wps = ps_p.tile([D, 512 * np_max], F32, tag="pp",
                            name=f"{_rp}wps")[:, 0:512]
            for i in range(8):
                nc.tensor.matmul(wps, warm[:, 0:128], warm,
                                 start=True, stop=True)

            def load_consts_tail():
                # issued after the first two doc jobs' DMAs: the query side
                # and chunk 4-7 weights aren't needed until stage_b(0) ends
                nc.sync.dma_start(out=qnat, in_=QE)
                nc.sync.dma_start(out=rt_raw[:, 512:NB], in_=RT[:, 512:NB])
                nc.sync.dma_start(out=rt_raw[:, NB:2 * NB], in_=RT[:, NB:2 * NB])
                nc.vector.tensor_copy(rhl[:, 512:NB], rt_raw[:, 512:NB])
                nc.vector.tensor_copy(rhl[:, NB:2 * NB], rt_raw[:, NB:2 * NB])

            U1 = const.tile([D, CH * QW], FP8, tag="U1", name=f"{_rp}U1")

            def query_proj():
                # exact 3-term compensated tf32 split: query-code flips cost
                # ~pad_c simmat entries each, so keep the query side exact
                qh = const.tile([D, QW], F32R, tag="qh", name=f"{_rp}qh")
                nc.vector.tensor_copy(qh, qnat)
                ql = const.tile([D, QW], F32R, tag="ql", name=f"{_rp}ql")
                nc.vector.tensor_tensor(ql, qnat, qh, Alu.subtract)
                for k in range(CH):
                    rh_k = rhl[:, k * 128:(k + 1) * 128]
                    rl_k = rhl[:, NB + k * 128:NB + (k + 1) * 128]
                    qp = ps_p.tile([D, 512 * np_max], F32, tag="pp",
                                   name=f"{_rp}qp{k}")[:, 0:QW]
                    nc.tensor.matmul(qp, rh_k, qh, start=True, stop=False)
                    nc.tensor.matmul(qp, rh_k, ql, start=False, stop=False)
                    nc.tensor.matmul(qp, rl_k, qh, start=False, stop=True)
                    u1k = U1[:, k * QW:(k + 1) * QW]
                    if k in DVE_CHUNKS:
                        nc.vector.tensor_scalar(u1k, qp, 1.0, -1.0,
                                                Alu.min, Alu.max)
                    else:
                        nc.scalar.activation(u1k, qp, Act.Sign)

            # ---- doc jobs, software-pipelined emission ----
            # stage A: dma (lands f32r directly); stage B: project+sign;
            # stage C: code dot + sin + dma out.  Emitting A(j+2)/B(j+1)
            # before C(j) lets the PE run projections while DVE/ACT finish
            # the previous job's signs.
            _slot_order = sorted(range(BPC), key=lambda s: -pads_c[s])
            jobs = [(b, l) for b in _slot_order for l in range(L)]
            st = [dict() for _ in jobs]

            def stage_a(j):
                b, l = jobs[j]
                pad_c = pads_c[b]
                e = jobp.tile([D, pad_cmax], F32R, tag="e",
                              name=f"{_rp}e{j}")[:, 0:pad_c]
                nc.sync.dma_start(out=e, in_=DE[b, l, :, 0:pad_c])
                st[j]["e"] = e

            def stage_b(j):
                b, l = jobs[j]
                pad_c = pads_c[b]
                splits = slot_splits[b]
                npieces = len(splits)
                e = st[j]["e"]
                U2 = jobp.tile([D, CH * pad_cmax], FP8, tag="U2",
                               name=f"{_rp}U2{j}")
                for k in range(CH):
                    rh_k = rhl[:, k * 128:(k + 1) * 128]
                    pp = ps_p.tile([D, 512 * np_max], F32, tag="pp",
                                   name=f"{_rp}pp{j}_{k}")
                    for c0, c1, p0 in splits:
                        nc.tensor.matmul(pp[:, p0:p0 + c1 - c0], rh_k,
                                         e[:, c0:c1], start=True, stop=True)
                    w = splits[0][1] - splits[0][0]
                    if npieces == 1 or npieces * w == pad_c:
                        if npieces == 1:
                            ppv = pp[:, 0:pad_c]
                            u2v = U2[:, k * pad_c:(k + 1) * pad_c]
                        else:
                            ppv = pp[:].rearrange("p (n c) -> p n c",
                                                  c=512)[:, 0:npieces, 0:w]
                            u2v = U2[:, k * pad_c:(k + 1) * pad_c] \
                                .rearrange("p (n c) -> p n c", c=w)
                        if k in DVE_CHUNKS:
                            nc.vector.tensor_scalar(u2v, ppv, 1.0, -1.0,
                                                    Alu.min, Alu.max)
                        else:
                            nc.scalar.activation(u2v, ppv, Act.Sign)
                    else:
                        for c0, c1, p0 in splits:
                            u2p = U2[:, k * pad_c + c0:k * pad_c + c1]
                            ppp = pp[:, p0:p0 + c1 - c0]
                            if k in DVE_CHUNKS:
                                nc.vector.tensor_scalar(u2p, ppp, 1.0, -1.0,
                                                        Alu.min, Alu.max)
                            else:
                                nc.scalar.activation(u2p, ppp, Act.Sign)
                st[j]["U2"] = U2

            def stage_c(j):
                b, l = jobs[j]
                pad_c = pads_c[b]
                splits = slot_splits[b]
                npieces = len(splits)
                U2 = st[j]["U2"]
                # code dot via fp8 DoubleRow: chunk pairs (2jj, 2jj+1) fold
                # into one K=256 matmul; +-1/0 codes are exact in fp8e4m3
                S = ps_p.tile([qpad, 512 * np_max], F32, tag="pp",
                              name=f"{_rp}S{j}")
                qcol = (b * L + l) * qpad
                for c0, c1, p0 in splits:
                    w = c1 - c0
                    for jj in range(CH // 2):
                        lw = U1[:, 2 * jj * QW:(2 * jj + 2) * QW] \
                            .rearrange("p (o c) -> p o c", o=2)[:, :, qcol:qcol + qpad]
                        rv = U2[:, 2 * jj * pad_c:(2 * jj + 2) * pad_c] \
                            .rearrange("p (o c) -> p o c", o=2)[:, :, c0:c1]
                        nc.tensor.matmul(
                            S[:, p0:p0 + w], lw, rv,
                            start=(jj == 0), stop=(jj == CH // 2 - 1),
                            perf_mode=mybir.MatmulPerfMode.DoubleRow,
                        )
                sim = outp.tile([qpad, pad_cmax], F32, tag="sim",
                                name=f"{_rp}sim{j}")[:, 0:pad_c]
                w = splits[0][1] - splits[0][0]
                if npieces > 1 and npieces * w == pad_c:
                    sv = S[:].rearrange("p (n c) -> p n c",
                                        c=512)[:, 0:npieces, 0:w]
                    mv = sim.rearrange("p (n c) -> p n c", c=w)
                    nc.scalar.activation(mv, sv, Act.Sin, scale=PI / (2.0 * NB))
                elif npieces == 1:
                    nc.scalar.activation(sim, S[:, 0:pad_c], Act.Sin,
                                         scale=PI / (2.0 * NB))
                else:
                    for c0, c1, p0 in splits:
                        nc.scalar.activation(sim[:, c0:c1], S[:, p0:p0 + c1 - c0],
                                             Act.Sin, scale=PI / (2.0 * NB))
                nc.sync.dma_start(out=OUT[b, l, :, 0:pad_c], in_=sim)

            n = len(jobs)
            stage_a(0)
            if n > 1:
                stage_a(1)
            load_consts_tail()
            stage_b(0)
            query_proj()
            for j in range(n - 1):
                stage_c(j)
                if j + 2 < n:
                    stage_a(j + 2)
                stage_b(j + 1)
            stage_c(n - 1)

    nc.compile()
    return nc


def _tf32(x):
    """Round-to-nearest-even to 11-bit mantissa (bit-matches fp32_to_fp32r)."""
    u = np.ascontiguousarray(x, np.float32).view(np.uint32).astype(np.uint64)
    u = (u + 0x07FF + ((u >> 12) & 1)) & 0xFFFFFFFFFFFFF000
    return (u & 0xFFFFFFFF).astype(np.uint32).view(np.float32)


def _stage_inputs(query_embed, doc_embed, query_tok, doc_tok, r):
    query_embed = np.ascontiguousarray(query_embed, dtype=np.float32)
    doc_embed = np.ascontiguousarray(doc_embed, dtype=np.float32)
    r = np.ascontiguousarray(r, dtype=np.float32)

    qmask = (np.asarray(query_tok) != 0)
    dmask = (np.asarray(doc_tok) != 0)

    # sort batches by active count; slot s takes ranks [s*CORES, (s+1)*CORES)
    # spread across the 8 cores, so per-slot padding is tight and identical
    # on every core (SPMD requires one shape per slot)
    counts = dmask.sum(axis=1).astype(int)
    order = np.argsort(counts, kind="stable")
    assign = np.empty((CORES, BPC), dtype=int)   # assign[c, b] = batch id
    for s in range(BPC):
        for c in range(CORES):
            assign[c, s] = order[s * CORES + c]
    pads_c = tuple(
        min(BDOC, max(64, int(-(-int(counts[assign[:, s]].max()) // 32) * 32)))
        for s in range(BPC)
    )
    pad_cmax = max(pads_c)

    qe_m = query_embed * qmask[None, :, :, None].astype(np.float32)
    qidxs = [np.flatnonzero(qmask[g]) for g in range(BAT)]
    qpad = min(A, max(16, int(-(-max(len(q) for q in qidxs) // 16) * 16)))
    # r is pre-scaled and pre-split into tf32 hi/lo halves
    rts = np.ascontiguousarray(r.T * SCALE)
    rh_host = _tf32(rts)
    rl_host = _tf32(rts - rh_host)
    rt = np.ascontiguousarray(np.concatenate([rh_host, rl_host], axis=1))

    idxs = [np.flatnonzero(dmask[g]) for g in range(BAT)]
    in_maps = []
    for c in range(CORES):
        # embeddings staged pre-transposed [D, tokens]; queries compacted
        # to their active rows (masks are per-batch, shared by both layers);
        # doc embeddings pre-rounded to tf32 so the hw f32r read is exact
        qe_c = np.zeros((D, BPC * L * qpad), dtype=np.float32)
        de_c = np.zeros((BPC, L, D, pad_cmax), dtype=np.float32)
        for b in range(BPC):
            g = assign[c, b]
            qi = qidxs[g]
            for li in range(L):
                col = (b * L + li) * qpad
                qe_c[:, col:col + len(qi)] = qe_m[li, g, qi].T
            idx = idxs[g]
            de_c[b, :, :, :len(idx)] = doc_embed[:, g, idx].transpose(0, 2, 1)
        in_maps.append({"qe": qe_c, "de": _tf32(de_c), "rt": rt})

    return in_maps, assign, idxs, pads_c, qidxs, qpad


def kernel(query_embed, doc_embed, query_tok, doc_tok, r):
    in_maps, assign, idxs, pads_c, qidxs, qpad = _stage_inputs(
        query_embed, doc_embed, query_tok, doc_tok, r)

    key = (pads_c, qpad)
    if key not in _BUILD_CACHE:
        _BUILD_CACHE[key] = _build(pads_c, qpad)
    nc = _BUILD_CACHE[key]

    res = run_bass_kernel_spmd(nc, in_maps, core_ids=list(range(CORES)))

    out = np.zeros((BAT, L, A, BDOC), dtype=np.float32)
    for c in range(CORES):
        o_c = res.results[c]["out"]  # [BPC, L, qpad, pad_cmax]
        for b in range(BPC):
            g = assign[c, b]
            idx = idxs[g]
            qi = qidxs[g]
            for li in range(L):
                out[g, li][np.ix_(qi, idx)] = o_c[b, li, :len(qi), :len(idx)]
    return out


# revision 12
# speedup vs baseline: 2094.2122x; 2094.2122x over previous
"""LSH similarity-matrix kernel for Trainium2 (8 NeuronCores, data-parallel over batch).

Math: reference computes, per (l, b):
    c1 = (query_embed @ r.T > 0),  c2 = (doc_embed @ r.T > 0)   in {0,1}
    ham = s1 + s2 - 2*c1@c2.T ;  sim = cos(pi/NB * ham), masked where tok==0.
With +-1 codes U = 2c-1 and S = U1 @ U2.T:  ham = (NB - S)/2, so
    sim = sin(pi/(2*NB) * S).
Masks fold into the embeddings: a zeroed embedding row projects to 0,
sign(0) = 0 gives a zero code row, so S = 0 and sin(0) = 0 — exactly the
masked output. Masked doc tokens (half of them: tok in {0,1}) are gathered
away host-side entirely; output columns scatter back as zeros. Batches are
assigned to (core, slot) sorted by active-token count so every core runs an
identically-shaped program with minimal padding per slot.

Structure per batch slot (2 layers):
  one input DMA  ([D, L*pad] doc embeddings, host layout [BPC, D, L, pad]),
  16 single-f32r projection matmuls (8 bit-chunks x 2 layers),
  16 sign ops split across ACT (Sign) / DVE (clamp) — one per chunk, each a
    single instruction over the strided PSUM pieces,
  8  fp8e4m3 DoubleRow code-dot matmuls into one S tile (layers at column
    offsets 0 / 512*np; DoubleRow requires dst partition base 0),
  1  Sin activation over both layers' S columns, fp16 output,
  1  output DMA (host layout [BPC, qpad, L, pad]).
The query side (tiny) runs once, first, so its signs land in the engine
idle window while the first doc DMAs are still in flight.

Precision: both projections run as a SINGLE float32r (TF32-class) matmul
per 128-bit chunk — 1 cycle/row vs fp32's 4 (moving dim >= 256). The f32r
rounding flips hash bits where |proj| falls below the rounding error;
measured on this benchmark data the resulting sim error is <1e-2 relative,
inside the 2e-2 gate. The code dot runs as fp8e4m3 DoubleRow matmuls
(chunk pairs give K=256 per MM at 2 MACs/cell/cycle); +-1/0 codes and
their fp32 PSUM accumulation are exact. sim in [-1,1] makes fp16 output
rounding (2^-11) negligible.

r is pre-scaled by 2^66 host-side so the DVE/gpsimd sign alternative
clamp(x, -1, 1) = max(min(x,1),-1) is exact (any |proj| > 2^-66 maps to
+-1). Embeddings are pre-rounded to tf32 host-side (free) and land via
DMA directly into f32r tiles, so no per-job DVE split/copy work remains.
"""
import os
import sys

sys.path.insert(0, "/opt/trn_rl_repo")

from contextlib import ExitStack

import numpy as np

import concourse.bass as bass
import concourse.mybir as mybir
import concourse.tile as tile
from concourse import bacc
from concourse.bass_utils import run_bass_kernel_spmd

L, BAT, A, BDOC, D, NB = 2, 32, 64, 1024, 128, 1024
CORES = 8
BPC = BAT // CORES          # batch slots per core
CH = NB // 128              # 8 bit-chunks
SCALE = float(2.0 ** 66)
PI = float(np.pi)

F32 = mybir.dt.float32
F32R = mybir.dt.float32r
F16 = mybir.dt.float16
BF16 = mybir.dt.bfloat16
FP8 = mybir.dt.float8e4
Alu = mybir.AluOpType
Act = mybir.ActivationFunctionType

_BUILD_CACHE: dict = {}


def _col_splits(n):
    """Split [0, n) into equal-width pieces of <=512 columns (>=256 keeps
    float32r matmuls at full rate; a matmul may not cross a PSUM bank, so
    piece i is written at PSUM column 512*i). Equal widths mean one strided
    [p, npieces, w] access pattern covers all pieces, so signs run as a
    single instruction per chunk. Returns (c0, c1, p0) per piece."""
    npieces = -(-n // 512)
    w = -(-(n // npieces) // 8) * 8
    while w * npieces < n:
        w += 8
    assert w * npieces >= n and w <= 512
    return [(i * w, min((i + 1) * w, n), 512 * i) for i in range(npieces)]


def _sign(eng, out_ap, in_ap, nc):
    if eng == "dve":
        nc.vector.tensor_scalar(out_ap, in_ap, 1.0, -1.0, Alu.min, Alu.max)
    else:
        nc.scalar.activation(out_ap, in_ap, Act.Sign)


def _chunk_engine(l, k):
    """Per-(layer, chunk) sign engine: 'dve' | 'act' | 'split' (pieces go
    half to DVE, half to ACT). Balances ACT (which also runs the per-slot
    Sin) against DVE: per slot DVE gets 8 chunks + half, ACT 7 + half."""
    if k in (0, 2, 4, 6):
        return "dve"
    if l == 1 and k == 7:
        return "split"
    return "act"


def _build(pads_c: tuple, qpad: int = A, reps: int = 1, warm_n: int = 4):
    """Per-core SPMD program. pads_c[b]: compute width (mult of 8) of batch
    slot b. reps > 1 re-emits the whole body (timing instrumentation only)."""
    pads_c = tuple(int(p) for p in pads_c)
    pad_cmax = max(pads_c)
    slot_splits = [_col_splits(p) for p in pads_c]
    np_max = max(len(s) for s in slot_splits)

    nc = bacc.Bacc("TRN2", target_bir_lowering=False, debug=False)

    QW = BPC * L * qpad
    QE = nc.dram_tensor("qe", [D, QW], F32R, kind="ExternalInput").ap()
    DE = nc.dram_tensor("de", [BPC, D, L, pad_cmax], F32R, kind="ExternalInput").ap()
    RT = nc.dram_tensor("rt", [D, NB], F32R, kind="ExternalInput").ap()
    OUT = nc.dram_tensor("out", [BPC, qpad, L, pad_cmax], F16,
                         kind="ExternalOutput").ap()

    with tile.TileContext(nc) as tc, ExitStack() as ctx:
        const = ctx.enter_context(tc.tile_pool(name="const", bufs=1))
        jobp = ctx.enter_context(tc.tile_pool(name="jobp", bufs=3))
        outp = ctx.enter_context(tc.tile_pool(name="outp", bufs=2))
        ps_p = ctx.enter_context(
            tc.tile_pool(name="ps_p", bufs=(8 - 2 * np_max) // np_max,
                         space="PSUM"))
        ps_s = ctx.enter_context(tc.tile_pool(name="ps_s", bufs=1,
                                              space="PSUM"))

        for _rep in range(reps):
            _rp = f"r{_rep}_"
            rh = const.tile([D, NB], F32R, tag="rh", name=f"{_rp}rh")
            qe = const.tile([D, QW], F32R, tag="qe", name=f"{_rp}qe")
            U1 = const.tile([D, CH * QW], FP8, tag="U1", name=f"{_rp}U1")

            _slot_order = sorted(range(BPC), key=lambda s: -pads_c[s])
            st = [dict() for _ in range(BPC)]

            # constants + first two slots' doc DMAs, all issued up front
            nc.sync.dma_start(out=rh[:, 0:512], in_=RT[:, 0:512])
            nc.sync.dma_start(out=qe, in_=QE)
            nc.sync.dma_start(out=rh[:, 512:NB], in_=RT[:, 512:NB])

            def stage_a(si):
                b = _slot_order[si]
                pad_c = pads_c[b]
                e = jobp.tile([D, L * pad_cmax], F32R, tag="e",
                              name=f"{_rp}e{si}")
                nc.sync.dma_start(
                    out=e[:].rearrange("p (o c) -> p o c",
                                       o=L)[:, :, 0:pad_c],
                    in_=DE[b, :, :, 0:pad_c])
                st[si]["e"] = e

            stage_a(0)
            if BPC > 1:
                stage_a(1)

            # PE pre-warm: dependency-free dummy matmuls pull the PE through
            # its cold/mid clock ramp while the first DMAs land
            if warm_n:
                warm = const.tile([D, 512], BF16, tag="warm", name=f"{_rp}warm")
                nc.vector.memset(warm, 0.0)
                wps = ps_p.tile([D, 512 * np_max], F32, tag="pp",
                                name=f"{_rp}wps")[:, 0:512]
                for i in range(warm_n):
                    nc.tensor.matmul(wps, warm[:, 0:128], warm,
                                     start=True, stop=True)

            def query_proj():
                # runs first: its signs fill the ACT/DVE idle window while
                # the doc DMAs are still landing
                for k in range(CH):
                    rh_k = rh[:, k * 128:(k + 1) * 128]
                    qp = ps_p.tile([D, 512 * np_max], F32, tag="pp",
                                   name=f"{_rp}qp{k}")[:, 0:QW]
                    nc.tensor.matmul(qp, rh_k, qe, start=True, stop=True)
                    u1k = U1[:, k * QW:(k + 1) * QW]
                    _sign("dve" if k % 2 == 0 else "act", u1k, qp, nc)

            def stage_b(si, l):
                # projection + sign for one (slot, layer): 8 chunk matmuls,
                # 8 sign instructions (engine per _chunk_engine)
                b = _slot_order[si]
                pad_c = pads_c[b]
                splits = slot_splits[b]
                npieces = len(splits)
                e = st[si]["e"][:].rearrange(
                    "p (o c) -> p o c", o=L)[:, l, :][:, 0:pad_c]
                if l == 0:
                    st[si]["U2"] = jobp.tile([D, L * CH * pad_cmax], FP8,
                                             tag="U2", name=f"{_rp}U2{si}")
                U2 = st[si]["U2"][:, l * CH * pad_cmax:(l + 1) * CH * pad_cmax]
                for k in range(CH):
                    rh_k = rh[:, k * 128:(k + 1) * 128]
                    pp = ps_p.tile([D, 512 * np_max], F32, tag="pp",
                                   name=f"{_rp}pp{si}_{l}_{k}")
                    for c0, c1, p0 in splits:
                        nc.tensor.matmul(pp[:, p0:p0 + c1 - c0], rh_k,
                                         e[:, c0:c1], start=True, stop=True)
                    eng = _chunk_engine(l, k)
                    w = splits[0][1] - splits[0][0]
                    u2c = U2[:, k * pad_c:(k + 1) * pad_c]
                    if eng == "split":
                        # halves alternate DVE/ACT to fine-balance the slot
                        if npieces == 1:
                            h = (pad_c // 16) * 8
                            _sign("dve", u2c[:, 0:h], pp[:, 0:h], nc)
                            _sign("act", u2c[:, h:pad_c], pp[:, h:pad_c], nc)
                        else:
                            for pi, (c0, c1, p0) in enumerate(splits):
                                _sign("dve" if pi % 2 == 0 else "act",
                                      u2c[:, c0:c1], pp[:, p0:p0 + c1 - c0], nc)
                    elif npieces == 1:
                        _sign(eng, u2c, pp[:, 0:pad_c], nc)
                    elif npieces * w == pad_c:
                        ppv = pp[:].rearrange("p (n c) -> p n c",
                                              c=512)[:, 0:npieces, 0:w]
                        u2v = u2c.rearrange("p (n c) -> p n c", c=w)
                        _sign(eng, u2v, ppv, nc)
                    else:
                        for c0, c1, p0 in splits:
                            _sign(eng, u2c[:, c0:c1],
                                  pp[:, p0:p0 + c1 - c0], nc)

            def stage_c(si, l):
                b = _slot_order[si]
                pad_c = pads_c[b]
                splits = slot_splits[b]
                U2 = st[si]["U2"][:, l * CH * pad_cmax:(l + 1) * CH * pad_cmax]
                # code dot via fp8 DoubleRow into the slot's shared S tile;
                # layer l occupies columns [l*512*np_max, ...): partition
                # base stays 0 as DoubleRow requires
                if l == 0:
                    S = ps_s.tile([qpad, L * 512 * np_max], F32, tag="S",
                                  name=f"{_rp}S{si}")
                    st[si]["S"] = S
                S = st[si]["S"]
                loff = l * 512 * np_max
                qcol = (b * L + l) * qpad
                for c0, c1, p0 in splits:
                    w = c1 - c0
                    for jj in range(CH // 2):
                        lw = U1[:, 2 * jj * QW:(2 * jj + 2) * QW] \
                            .rearrange("p (o c) -> p o c", o=2)[:, :, qcol:qcol + qpad]
                        rv = U2[:, 2 * jj * pad_c:(2 * jj + 2) * pad_c] \
                            .rearrange("p (o c) -> p o c", o=2)[:, :, c0:c1]
                        nc.tensor.matmul(
                            S[:, loff + p0:loff + p0 + w], lw, rv,
                            start=(jj == 0), stop=(jj == CH // 2 - 1),
                            perf_mode=mybir.MatmulPerfMode.DoubleRow,
                        )

            def stage_d(si):
                # one Sin per slot over both layers' S columns (fp16 out),
                # one output DMA per slot
                b = _slot_order[si]
                pad_c = pads_c[b]
                splits = slot_splits[b]
                npieces = len(splits)
                S = st[si]["S"]
                sim = outp.tile([qpad, L * pad_cmax], F16, tag="sim",
                                name=f"{_rp}sim{si}")
                w = splits[0][1] - splits[0][0]
                if npieces * w == pad_c or npieces == 1:
                    sv = S[:].rearrange("p (o n c) -> p o n c", o=L,
                                        c=512)[:, :, 0:npieces, 0:w]
                    mv = sim[:].rearrange("p (o c) -> p o c",
                                          o=L)[:, :, 0:pad_c] \
                        .rearrange("p o (n c) -> p o n c", c=w)
                    nc.scalar.activation(mv, sv, Act.Sin, scale=PI / (2.0 * NB))
                else:
                    for l_ in range(L):
                        for c0, c1, p0 in splits:
                            nc.scalar.activation(
                                sim[:, l_ * pad_cmax + c0:l_ * pad_cmax + c1],
                                S[:, l_ * 512 * np_max + p0:
                                  l_ * 512 * np_max + p0 + c1 - c0],
                                Act.Sin, scale=PI / (2.0 * NB))
                ov = OUT[b, :, :, 0:pad_c]
                sv2 = sim[:].rearrange("p (o c) -> p o c", o=L)[:, :, 0:pad_c]
                nc.sync.dma_start(out=ov, in_=sv2)

            # ---- emission: query first, then slot-pipelined doc stream ----
            query_proj()
            stage_b(0, 0)
            stage_b(0, 1)
            for si in range(BPC):
                stage_c(si, 0)
                stage_c(si, 1)
                stage_d(si)
                if si + 2 < BPC:
                    stage_a(si + 2)
                if si + 1 < BPC:
                    stage_b(si + 1, 0)
                    stage_b(si + 1, 1)

    nc.compile()
    return nc


def _tf32(x):
    """Round-to-nearest-even to 11-bit mantissa (bit-matches fp32_to_fp32r)."""
    u = np.ascontiguousarray(x, np.float32).view(np.uint32).astype(np.uint64)
    u = (u + 0x07FF + ((u >> 12) & 1)) & 0xFFFFFFFFFFFFF000
    return (u & 0xFFFFFFFF).astype(np.uint32).view(np.float32)


def _stage_inputs(query_embed, doc_embed, query_tok, doc_tok, r):
    query_embed = np.ascontiguousarray(query_embed, dtype=np.float32)
    doc_embed = np.ascontiguousarray(doc_embed, dtype=np.float32)
    r = np.ascontiguousarray(r, dtype=np.float32)

    qmask = (np.asarray(query_tok) != 0)
    dmask = (np.asarray(doc_tok) != 0)

    # sort batches by active count; slot s takes ranks [s*CORES, (s+1)*CORES)
    # spread across the 8 cores, so per-slot padding is tight and identical
    # on every core (SPMD requires one shape per slot)
    counts = dmask.sum(axis=1).astype(int)
    order = np.argsort(counts, kind="stable")
    assign = np.empty((CORES, BPC), dtype=int)   # assign[c, b] = batch id
    for s in range(BPC):
        for c in range(CORES):
            assign[c, s] = order[s * CORES + c]
    pads_c = tuple(
        min(BDOC, max(64, int(-(-int(counts[assign[:, s]].max()) // 16) * 16)))
        for s in range(BPC)
    )
    pad_cmax = max(pads_c)

    qe_m = query_embed * qmask[None, :, :, None].astype(np.float32)
    qidxs = [np.flatnonzero(qmask[g]) for g in range(BAT)]
    qpad = min(A, max(16, int(-(-max(len(q) for q in qidxs) // 8) * 8)))
    # r is pre-scaled and pre-rounded to tf32 (hw f32r read is then exact)
    rt = _tf32(np.ascontiguousarray(r.T * SCALE))

    idxs = [np.flatnonzero(dmask[g]) for g in range(BAT)]
    in_maps = []
    for c in range(CORES):
        # embeddings staged pre-transposed [D, tokens]; queries compacted
        # to their active rows (masks are per-batch, shared by both layers);
        # embeddings pre-rounded to tf32 so the hw f32r read is exact
        qe_c = np.zeros((D, BPC * L * qpad), dtype=np.float32)
        de_c = np.zeros((BPC, D, L, pad_cmax), dtype=np.float32)
        for b in range(BPC):
            g = assign[c, b]
            qi = qidxs[g]
            for li in range(L):
                col = (b * L + li) * qpad
                qe_c[:, col:col + len(qi)] = qe_m[li, g, qi].T
            idx = idxs[g]
            de_c[b, :, :, :len(idx)] = doc_embed[:, g, idx].transpose(2, 0, 1)
        in_maps.append({"qe": _tf32(qe_c), "de": _tf32(de_c), "rt": rt})

    return in_maps, assign, idxs, pads_c, qidxs, qpad


def kernel(query_embed, doc_embed, query_tok, doc_tok, r):
    in_maps, assign, idxs, pads_c, qidxs, qpad = _stage_inputs(
        query_embed, doc_embed, query_tok, doc_tok, r)

    key = (pads_c, qpad)
    if key not in _BUILD_CACHE:
        _BUILD_CACHE[key] = _build(pads_c, qpad)
    nc = _BUILD_CACHE[key]

    res = run_bass_kernel_spmd(nc, in_maps, core_ids=list(range(CORES)))

    out = np.zeros((BAT, L, A, BDOC), dtype=np.float32)
    for c in range(CORES):
        o_c = res.results[c]["out"]  # [BPC, qpad, L, pad_cmax] fp16 sim
        for b in range(BPC):
            g = assign[c, b]
            idx = idxs[g]
            qi = qidxs[g]
            for li in range(L):
                out[g, li][np.ix_(qi, idx)] = o_c[b, :len(qi), li, :len(idx)]
    return out


# revision 13
# speedup vs baseline: 2115.3390x; 1.0101x over previous
"""LSH similarity-matrix kernel for Trainium2 (8 NeuronCores, data-parallel over batch).

Math: reference computes, per (l, b):
    c1 = (query_embed @ r.T > 0),  c2 = (doc_embed @ r.T > 0)   in {0,1}
    ham = s1 + s2 - 2*c1@c2.T ;  sim = cos(pi/NB * ham), masked where tok==0.
With +-1 codes U = 2c-1 and S = U1 @ U2.T:  ham = (NB - S)/2, so
    sim = sin(pi/(2*NB) * S).
Masks fold into the embeddings: a zeroed embedding row projects to 0,
sign(0) = 0 gives a zero code row, so S = 0 and sin(0) = 0 — exactly the
masked output. Masked doc tokens (half of them: tok in {0,1}) are gathered
away host-side entirely; output columns scatter back as zeros. Batches are
assigned to (core, slot) sorted by active-token count so every core runs an
identically-shaped program with minimal padding per slot.

Structure per batch slot (2 layers):
  one input DMA  ([D, L*pad] doc embeddings, host layout [BPC, D, L, pad]),
  16 single-f32r projection matmuls (8 bit-chunks x 2 layers),
  16 sign ops split across ACT (Sign) / DVE (clamp) — one per chunk, each a
    single instruction over the strided PSUM pieces,
  8  fp8e4m3 DoubleRow code-dot matmuls into one S tile (layers at column
    offsets 0 / 512*np; DoubleRow requires dst partition base 0),
  1  Sin activation over both layers' S columns, fp16 output,
  1  output DMA (host layout [BPC, qpad, L, pad]).
The query side (tiny) runs once, first, so its signs land in the engine
idle window while the first doc DMAs are still in flight.

Precision: both projections run as a SINGLE float32r (TF32-class) matmul
per 128-bit chunk — 1 cycle/row vs fp32's 4 (moving dim >= 256). The f32r
rounding flips hash bits where |proj| falls below the rounding error;
measured on this benchmark data the resulting sim error is <1e-2 relative,
inside the 2e-2 gate. The code dot runs as fp8e4m3 DoubleRow matmuls
(chunk pairs give K=256 per MM at 2 MACs/cell/cycle); +-1/0 codes and
their fp32 PSUM accumulation are exact. sim in [-1,1] makes fp16 output
rounding (2^-11) negligible.

r is pre-scaled by 2^66 host-side so the DVE/gpsimd sign alternative
clamp(x, -1, 1) = max(min(x,1),-1) is exact (any |proj| > 2^-66 maps to
+-1). Embeddings are pre-rounded to tf32 host-side (free) and land via
DMA directly into f32r tiles, so no per-job DVE split/copy work remains.
"""
import os
import sys

sys.path.insert(0, "/opt/trn_rl_repo")

from contextlib import ExitStack

import numpy as np

import concourse.bass as bass
import concourse.mybir as mybir
import concourse.tile as tile
from concourse import bacc
from concourse.bass_utils import run_bass_kernel_spmd

L, BAT, A, BDOC, D, NB = 2, 32, 64, 1024, 128, 1024
CORES = 8
BPC = BAT // CORES          # batch slots per core
CH = NB // 128              # 8 bit-chunks
SCALE = float(2.0 ** 66)
PI = float(np.pi)

F32 = mybir.dt.float32
F32R = mybir.dt.float32r
F16 = mybir.dt.float16
BF16 = mybir.dt.bfloat16
FP8 = mybir.dt.float8e4
Alu = mybir.AluOpType
Act = mybir.ActivationFunctionType

_BUILD_CACHE: dict = {}


def _col_splits(n):
    """Split [0, n) into equal-width pieces of <=512 columns (>=256 keeps
    float32r matmuls at full rate; a matmul may not cross a PSUM bank, so
    piece i is written at PSUM column 512*i). Equal widths mean one strided
    [p, npieces, w] access pattern covers all pieces, so signs run as a
    single instruction per chunk. Returns (c0, c1, p0) per piece."""
    npieces = -(-n // 512)
    w = -(-(n // npieces) // 8) * 8
    while w * npieces < n:
        w += 8
    assert w * npieces >= n and w <= 512
    return [(i * w, min((i + 1) * w, n), 512 * i) for i in range(npieces)]


def _sign(eng, out_ap, in_ap, nc):
    if eng == "dve":
        nc.vector.tensor_scalar(out_ap, in_ap, 1.0, -1.0, Alu.min, Alu.max)
    else:
        nc.scalar.activation(out_ap, in_ap, Act.Sign)


def _chunk_engine(l, k):
    """Per-(layer, chunk) sign engine: 'dve' | 'act' | 'split' (pieces go
    half to DVE, half to ACT). Balances ACT (which also runs the per-slot
    Sin) against DVE: per slot DVE gets 8 chunks + half, ACT 7 + half."""
    if k in (0, 2, 4, 6):
        return "dve"
    if l == 1 and k == 7:
        return "split"
    return "act"


def _build(pads_c: tuple, qpad: int = A, reps: int = 1, warm_n: int = 4):
    """Per-core SPMD program. pads_c[b]: compute width (mult of 8) of batch
    slot b. reps > 1 re-emits the whole body (timing instrumentation only)."""
    pads_c = tuple(int(p) for p in pads_c)
    pad_cmax = max(pads_c)
    slot_splits = [_col_splits(p) for p in pads_c]
    np_max = max(len(s) for s in slot_splits)

    nc = bacc.Bacc("TRN2", target_bir_lowering=False, debug=False)

    QW = BPC * L * qpad
    QE = nc.dram_tensor("qe", [D, QW], F32R, kind="ExternalInput").ap()
    DE = nc.dram_tensor("de", [BPC, D, L, pad_cmax], F32R, kind="ExternalInput").ap()
    RT = nc.dram_tensor("rt", [D, NB], F32R, kind="ExternalInput").ap()
    OUT = nc.dram_tensor("out", [BPC, qpad, L, pad_cmax], F16,
                         kind="ExternalOutput").ap()

    with tile.TileContext(nc) as tc, ExitStack() as ctx:
        const = ctx.enter_context(tc.tile_pool(name="const", bufs=1))
        jobp = ctx.enter_context(tc.tile_pool(name="jobp", bufs=3))
        outp = ctx.enter_context(tc.tile_pool(name="outp", bufs=2))
        ps_p = ctx.enter_context(
            tc.tile_pool(name="ps_p", bufs=(8 - 2 * np_max) // np_max,
                         space="PSUM"))
        ps_s = ctx.enter_context(tc.tile_pool(name="ps_s", bufs=1,
                                              space="PSUM"))

        for _rep in range(reps):
            _rp = f"r{_rep}_"
            rh = const.tile([D, NB], F32R, tag="rh", name=f"{_rp}rh")
            qe = const.tile([D, QW], F32R, tag="qe", name=f"{_rp}qe")
            U1 = const.tile([D, CH * QW], FP8, tag="U1", name=f"{_rp}U1")

            _slot_order = sorted(range(BPC), key=lambda s: -pads_c[s])
            st = [dict() for _ in range(BPC)]

            # constants + first two slots' doc DMAs, all issued up front
            nc.sync.dma_start(out=rh[:, 0:512], in_=RT[:, 0:512])
            nc.sync.dma_start(out=qe, in_=QE)
            nc.sync.dma_start(out=rh[:, 512:NB], in_=RT[:, 512:NB])

            def stage_a(si):
                b = _slot_order[si]
                pad_c = pads_c[b]
                e = jobp.tile([D, L * pad_cmax], F32R, tag="e",
                              name=f"{_rp}e{si}")
                nc.sync.dma_start(
                    out=e[:].rearrange("p (o c) -> p o c",
                                       o=L)[:, :, 0:pad_c],
                    in_=DE[b, :, :, 0:pad_c])
                st[si]["e"] = e

            stage_a(0)
            if BPC > 1:
                stage_a(1)

            # PE pre-warm: dependency-free dummy matmuls pull the PE through
            # its cold/mid clock ramp while the first DMAs land
            if warm_n:
                warm = const.tile([D, 512], BF16, tag="warm", name=f"{_rp}warm")
                nc.vector.memset(warm, 0.0)
                wps = ps_p.tile([D, 512 * np_max], F32, tag="pp",
                                name=f"{_rp}wps")[:, 0:512]
                for i in range(warm_n):
                    nc.tensor.matmul(wps, warm[:, 0:128], warm,
                                     start=True, stop=True)

            def query_proj():
                # runs first: its signs fill the ACT/DVE idle window while
                # the doc DMAs are still landing
                for k in range(CH):
                    rh_k = rh[:, k * 128:(k + 1) * 128]
                    qp = ps_p.tile([D, 512 * np_max], F32, tag="pp",
                                   name=f"{_rp}qp{k}")[:, 0:QW]
                    nc.tensor.matmul(qp, rh_k, qe, start=True, stop=True)
                    u1k = U1[:, k * QW:(k + 1) * QW]
                    _sign("dve" if k % 2 == 0 else "act", u1k, qp, nc)

            def stage_b(si, l):
                # projection + sign for one (slot, layer): 8 chunk matmuls,
                # 8 sign instructions (engine per _chunk_engine)
                b = _slot_order[si]
                pad_c = pads_c[b]
                splits = slot_splits[b]
                npieces = len(splits)
                e = st[si]["e"][:].rearrange(
                    "p (o c) -> p o c", o=L)[:, l, :][:, 0:pad_c]
                if l == 0:
                    st[si]["U2"] = jobp.tile([D, L * CH * pad_cmax], FP8,
                                             tag="U2", name=f"{_rp}U2{si}")
                U2 = st[si]["U2"][:, l * CH * pad_cmax:(l + 1) * CH * pad_cmax]
                for k in range(CH):
                    rh_k = rh[:, k * 128:(k + 1) * 128]
                    pp = ps_p.tile([D, 512 * np_max], F32, tag="pp",
                                   name=f"{_rp}pp{si}_{l}_{k}")
                    for c0, c1, p0 in splits:
                        nc.tensor.matmul(pp[:, p0:p0 + c1 - c0], rh_k,
                                         e[:, c0:c1], start=True, stop=True)
                    eng = _chunk_engine(l, k)
                    w = splits[0][1] - splits[0][0]
                    u2c = U2[:, k * pad_c:(k + 1) * pad_c]
                    if eng == "split":
                        # halves alternate DVE/ACT to fine-balance the slot
                        if npieces == 1:
                            h = (pad_c // 16) * 8
                            _sign("dve", u2c[:, 0:h], pp[:, 0:h], nc)
                            _sign("act", u2c[:, h:pad_c], pp[:, h:pad_c], nc)
                        else:
                            for pi, (c0, c1, p0) in enumerate(splits):
                                _sign("dve" if pi % 2 == 0 else "act",
                                      u2c[:, c0:c1], pp[:, p0:p0 + c1 - c0], nc)
                    elif npieces == 1:
                        _sign(eng, u2c, pp[:, 0:pad_c], nc)
                    elif npieces * w == pad_c:
                        ppv = pp[:].rearrange("p (n c) -> p n c",
                                              c=512)[:, 0:npieces, 0:w]
                        u2v = u2c.rearrange("p (n c) -> p n c", c=w)
                        _sign(eng, u2v, ppv, nc)
                    else:
                        for c0, c1, p0 in splits:
                            _sign(eng, u2c[:, c0:c1],
                                  pp[:, p0:p0 + c1 - c0], nc)

            def stage_c(si, l):
                b = _slot_order[si]
                pad_c = pads_c[b]
                splits = slot_splits[b]
                U2 = st[si]["U2"][:, l * CH * pad_cmax:(l + 1) * CH * pad_cmax]
                # code dot via fp8 DoubleRow into the slot's shared S tile;
                # layer l occupies columns [l*512*np_max, ...): partition
                # base stays 0 as DoubleRow requires
                if l == 0:
                    S = ps_s.tile([qpad, L * 512 * np_max], F32, tag="S",
                                  name=f"{_rp}S{si}")
                    st[si]["S"] = S
                S = st[si]["S"]
                loff = l * 512 * np_max
                qcol = (b * L + l) * qpad
                for c0, c1, p0 in splits:
                    w = c1 - c0
                    for jj in range(CH // 2):
                        lw = U1[:, 2 * jj * QW:(2 * jj + 2) * QW] \
                            .rearrange("p (o c) -> p o c", o=2)[:, :, qcol:qcol + qpad]
                        rv = U2[:, 2 * jj * pad_c:(2 * jj + 2) * pad_c] \
                            .rearrange("p (o c) -> p o c", o=2)[:, :, c0:c1]
                        nc.tensor.matmul(
                            S[:, loff + p0:loff + p0 + w], lw, rv,
                            start=(jj == 0), stop=(jj == CH // 2 - 1),
                            perf_mode=mybir.MatmulPerfMode.DoubleRow,
                        )

            def stage_d(si):
                # one Sin per slot over both layers' S columns (fp16 out),
                # one output DMA per slot
                b = _slot_order[si]
                pad_c = pads_c[b]
                splits = slot_splits[b]
                npieces = len(splits)
                S = st[si]["S"]
                sim = outp.tile([qpad, L * pad_cmax], F16, tag="sim",
                                name=f"{_rp}sim{si}")
                w = splits[0][1] - splits[0][0]
                if npieces * w == pad_c or npieces == 1:
                    sv = S[:].rearrange("p (o n c) -> p o n c", o=L,
                                        c=512)[:, :, 0:npieces, 0:w]
                    mv = sim[:].rearrange("p (o c) -> p o c",
                                          o=L)[:, :, 0:pad_c] \
                        .rearrange("p o (n c) -> p o n c", c=w)
                    nc.scalar.activation(mv, sv, Act.Sin, scale=PI / (2.0 * NB))
                else:
                    for l_ in range(L):
                        for c0, c1, p0 in splits:
                            nc.scalar.activation(
                                sim[:, l_ * pad_cmax + c0:l_ * pad_cmax + c1],
                                S[:, l_ * 512 * np_max + p0:
                                  l_ * 512 * np_max + p0 + c1 - c0],
                                Act.Sin, scale=PI / (2.0 * NB))
                ov = OUT[b, :, :, 0:pad_c]
                sv2 = sim[:].rearrange("p (o c) -> p o c", o=L)[:, :, 0:pad_c]
                nc.sync.dma_start(out=ov, in_=sv2)

            # ---- emission: query first, then slot-pipelined doc stream.
            # b(si+1) is emitted BEFORE c(si)/d(si): per-engine streams run
            # in emission order, so the PE projects slot si+1 while ACT/DVE
            # drain slot si's signs, and the ACT sin for slot si queues
            # behind slot si+1's signs instead of blocking them. ----
            query_proj()
            stage_b(0, 0)
            stage_b(0, 1)
            for si in range(BPC):
                if si + 2 < BPC:
                    stage_a(si + 2)
                if si + 1 < BPC:
                    stage_b(si + 1, 0)
                    stage_b(si + 1, 1)
                stage_c(si, 0)
                stage_c(si, 1)
                stage_d(si)

    nc.compile()
    return nc


def _tf32(x):
    """Round-to-nearest-even to 11-bit mantissa (bit-matches fp32_to_fp32r)."""
    u = np.ascontiguousarray(x, np.float32).view(np.uint32).astype(np.uint64)
    u = (u + 0x07FF + ((u >> 12) & 1)) & 0xFFFFFFFFFFFFF000
    return (u & 0xFFFFFFFF).astype(np.uint32).view(np.float32)


def _stage_inputs(query_embed, doc_embed, query_tok, doc_tok, r):
    query_embed = np.ascontiguousarray(query_embed, dtype=np.float32)
    doc_embed = np.ascontiguousarray(doc_embed, dtype=np.float32)
    r = np.ascontiguousarray(r, dtype=np.float32)

    qmask = (np.asarray(query_tok) != 0)
    dmask = (np.asarray(doc_tok) != 0)

    # sort batches by active count; slot s takes ranks [s*CORES, (s+1)*CORES)
    # spread across the 8 cores, so per-slot padding is tight and identical
    # on every core (SPMD requires one shape per slot)
    counts = dmask.sum(axis=1).astype(int)
    order = np.argsort(counts, kind="stable")
    assign = np.empty((CORES, BPC), dtype=int)   # assign[c, b] = batch id
    for s in range(BPC):
        for c in range(CORES):
            assign[c, s] = order[s * CORES + c]
    pads_c = tuple(
        min(BDOC, max(64, int(-(-int(counts[assign[:, s]].max()) // 16) * 16)))
        for s in range(BPC)
    )
    pad_cmax = max(pads_c)

    qe_m = query_embed * qmask[None, :, :, None].astype(np.float32)
    qidxs = [np.flatnonzero(qmask[g]) for g in range(BAT)]
    qpad = min(A, max(16, int(-(-max(len(q) for q in qidxs) // 8) * 8)))
    # r is pre-scaled and pre-rounded to tf32 (hw f32r read is then exact)
    rt = _tf32(np.ascontiguousarray(r.T * SCALE))

    idxs = [np.flatnonzero(dmask[g]) for g in range(BAT)]
    in_maps = []
    for c in range(CORES):
        # embeddings staged pre-transposed [D, tokens]; queries compacted
        # to their active rows (masks are per-batch, shared by both layers);
        # embeddings pre-rounded to tf32 so the hw f32r read is exact
        qe_c = np.zeros((D, BPC * L * qpad), dtype=np.float32)
        de_c = np.zeros((BPC, D, L, pad_cmax), dtype=np.float32)
        for b in range(BPC):
            g = assign[c, b]
            qi = qidxs[g]
            for li in range(L):
                col = (b * L + li) * qpad
                qe_c[:, col:col + len(qi)] = qe_m[li, g, qi].T
            idx = idxs[g]
            de_c[b, :, :, :len(idx)] = doc_embed[:, g, idx].transpose(2, 0, 1)
        in_maps.append({"qe": _tf32(qe_c), "de": _tf32(de_c), "rt": rt})

    return in_maps, assign, idxs, pads_c, qidxs, qpad


def kernel(query_embed, doc_embed, query_tok, doc_tok, r):
    in_maps, assign, idxs, pads_c, qidxs, qpad = _stage_inputs(
        query_embed, doc_embed, query_tok, doc_tok, r)

    key = (pads_c, qpad)
    if key not in _BUILD_CACHE:
        _BUILD_CACHE[key] = _build(pads_c, qpad)
    nc = _BUILD_CACHE[key]

    res = run_bass_kernel_spmd(nc, in_maps, core_ids=list(range(CORES)))

    out = np.zeros((BAT, L, A, BDOC), dtype=np.float32)
    for c in range(CORES):
        o_c = res.results[c]["out"]  # [BPC, qpad, L, pad_cmax] fp16 sim
        for b in range(BPC):
            g = assign[c, b]
            idx = idxs[g]
            qi = qidxs[g]
            for li in range(L):
                out[g, li][np.ix_(qi, idx)] = o_c[b, :len(qi), li, :len(idx)]
    return out


# revision 19
# speedup vs baseline: 3039.7960x; 1.4370x over previous
"""LSH similarity-matrix kernel for Trainium2 (8 NeuronCores, data-parallel over batch).

Math: reference computes, per (l, b):
    c1 = (query_embed @ r.T > 0),  c2 = (doc_embed @ r.T > 0)   in {0,1}
    ham = s1 + s2 - 2*c1@c2.T ;  sim = cos(pi/NB * ham), masked where tok==0.
With +-1 codes U = 2c-1 and S = U1 @ U2.T:  ham = (NB - S)/2, so
    sim = sin(pi/(2*NB) * S).
Masks fold into the embeddings: a zeroed embedding row projects to 0,
sign(0) = 0 gives a zero code row, so S = 0 and sin(0) = 0 — exactly the
masked output. Masked doc tokens (half of them: tok in {0,1}) are gathered
away host-side entirely; output columns scatter back as zeros. Batches are
assigned to (core, slot) sorted by active-token count so every core runs an
identically-shaped program with minimal padding per slot.

Structure per batch slot (2 layers):
  one input DMA  ([D, L*pad] doc embeddings, host layout [BPC, D, L, pad]),
  16 single-f32r projection matmuls (8 bit-chunks x 2 layers),
  16 sign ops split across ACT (Sign) / DVE (clamp) — one per chunk, each a
    single instruction over the strided PSUM pieces,
  8  fp8e4m3 DoubleRow code-dot matmuls into one S tile (layers at column
    offsets 0 / 512*np; DoubleRow requires dst partition base 0),
  1  Sin activation over both layers' S columns, fp16 output,
  1  output DMA (host layout [BPC, qpad, L, pad]).
The query side (tiny) runs once, first, so its signs land in the engine
idle window while the first doc DMAs are still in flight.

Precision: both projections run as a SINGLE float32r (TF32-class) matmul
per 128-bit chunk — 1 cycle/row vs fp32's 4 (moving dim >= 256). The f32r
rounding flips hash bits where |proj| falls below the rounding error;
measured on this benchmark data the resulting sim error is <1e-2 relative,
inside the 2e-2 gate. The code dot runs as fp8e4m3 DoubleRow matmuls
(chunk pairs give K=256 per MM at 2 MACs/cell/cycle); +-1/0 codes and
their fp32 PSUM accumulation are exact. sim in [-1,1] makes fp16 output
rounding (2^-11) negligible.

r is pre-scaled by 2^66 host-side so the DVE/gpsimd sign alternative
clamp(x, -1, 1) = max(min(x,1),-1) is exact (any |proj| > 2^-66 maps to
+-1). Embeddings are pre-rounded to tf32 host-side (free) and land via
DMA directly into f32r tiles, so no per-job DVE split/copy work remains.
"""
import os
import sys

sys.path.insert(0, "/opt/trn_rl_repo")

from contextlib import ExitStack

import numpy as np

import concourse.bass as bass
import concourse.mybir as mybir
import concourse.tile as tile
from concourse import bacc
from concourse.bass_utils import run_bass_kernel_spmd

L, BAT, A, BDOC, D, NB = 2, 32, 64, 1024, 128, 1024
CORES = 8
BPC = BAT // CORES          # batch slots per core
CH = NB // 128              # 8 bit-chunks
SCALE = float(2.0 ** 66)
PI = float(np.pi)

F32 = mybir.dt.float32
F32R = mybir.dt.float32r
F16 = mybir.dt.float16
BF16 = mybir.dt.bfloat16
FP8 = mybir.dt.float8e4
Alu = mybir.AluOpType
Act = mybir.ActivationFunctionType

_BUILD_CACHE: dict = {}


def _col_splits(n):
    """Split [0, n) into equal-width pieces of <=512 columns (>=256 keeps
    float32r matmuls at full rate; a matmul may not cross a PSUM bank, so
    piece i is written at PSUM column 512*i). Equal widths mean one strided
    [p, npieces, w] access pattern covers all pieces, so signs run as a
    single instruction per chunk. Returns (c0, c1, p0) per piece."""
    npieces = -(-n // 512)
    w = -(-(n // npieces) // 8) * 8
    while w * npieces < n:
        w += 8
    assert w * npieces >= n and w <= 512
    return [(i * w, min((i + 1) * w, n), 512 * i) for i in range(npieces)]


def _sign(eng, out_ap, in_ap, nc):
    if eng == "dve":
        nc.vector.tensor_scalar(out_ap, in_ap, 1.0, -1.0, Alu.min, Alu.max)
    else:
        nc.scalar.activation(out_ap, in_ap, Act.Sign)


def _chunk_engine(l, k):
    """Per-(layer, chunk) sign engine. Balances ACT (which also runs the
    two per-slot Sins) against DVE: per slot DVE gets 9 chunks, ACT 7."""
    if k in (0, 2, 4, 6) or (l == 1 and k == 7):
        return "dve"
    return "act"


def _build(pads_c: tuple, qpad: int = A, reps: int = 1, warm_n: int = 4):
    """Per-core SPMD program. pads_c[b]: compute width (mult of 8) of batch
    slot b. reps > 1 re-emits the whole body (timing instrumentation only)."""
    pads_c = tuple(int(p) for p in pads_c)
    pad_cmax = max(pads_c)
    slot_splits = [_col_splits(p) for p in pads_c]
    np_max = max(len(s) for s in slot_splits)

    nc = bacc.Bacc("TRN2", target_bir_lowering=False, debug=False)

    QW = BPC * L * qpad
    QE = nc.dram_tensor("qe", [D, QW], F32R, kind="ExternalInput").ap()
    DE = nc.dram_tensor("de", [BPC, D, L, pad_cmax], F32R, kind="ExternalInput").ap()
    RT = nc.dram_tensor("rt", [D, NB], F32R, kind="ExternalInput").ap()
    OUT = nc.dram_tensor("out", [BPC, qpad, L, pad_cmax], F16,
                         kind="ExternalOutput").ap()

    with tile.TileContext(nc) as tc, ExitStack() as ctx:
        const = ctx.enter_context(tc.tile_pool(name="const", bufs=1))
        jobp = ctx.enter_context(tc.tile_pool(name="jobp", bufs=3))
        outp = ctx.enter_context(tc.tile_pool(name="outp", bufs=2))
        # one rotating PSUM pool shared by projections, query and the code
        # dots: maximum pipeline depth (4 tiles x 2 banks), no dedicated
        # S banks sitting idle
        ps_p = ctx.enter_context(
            tc.tile_pool(name="ps_p", bufs=8 // np_max, space="PSUM"))

        for _rep in range(reps):
            _rp = f"r{_rep}_"
            rh = const.tile([D, NB], F32R, tag="rh", name=f"{_rp}rh")
            qe = const.tile([D, QW], F32R, tag="qe", name=f"{_rp}qe")
            U1 = const.tile([D, CH * QW], FP8, tag="U1", name=f"{_rp}U1")

            _slot_order = sorted(range(BPC), key=lambda s: -pads_c[s])
            st = [dict() for _ in range(BPC)]

            # constants + first two slots' doc DMAs, all issued up front
            nc.sync.dma_start(out=rh[:, 0:512], in_=RT[:, 0:512])
            nc.sync.dma_start(out=qe, in_=QE)
            nc.sync.dma_start(out=rh[:, 512:NB], in_=RT[:, 512:NB])

            def stage_a(si):
                b = _slot_order[si]
                pad_c = pads_c[b]
                e = jobp.tile([D, L * pad_cmax], F32R, tag="e",
                              name=f"{_rp}e{si}")
                nc.sync.dma_start(
                    out=e[:].rearrange("p (o c) -> p o c",
                                       o=L)[:, :, 0:pad_c],
                    in_=DE[b, :, :, 0:pad_c])
                st[si]["e"] = e

            stage_a(0)
            if BPC > 1:
                stage_a(1)

            # PE pre-warm: dependency-free dummy matmuls pull the PE through
            # its cold/mid clock ramp while the first DMAs land
            if warm_n:
                warm = const.tile([D, 512], BF16, tag="warm", name=f"{_rp}warm")
                nc.vector.memset(warm, 0.0)
                wps = ps_p.tile([D, 512 * np_max], F32, tag="pp",
                                name=f"{_rp}wps")[:, 0:512]
                for i in range(warm_n):
                    nc.tensor.matmul(wps, warm[:, 0:128], warm,
                                     start=True, stop=True)

            def query_proj():
                # runs first: its signs fill the ACT/DVE idle window while
                # the doc DMAs are still landing
                for k in range(CH):
                    rh_k = rh[:, k * 128:(k + 1) * 128]
                    qp = ps_p.tile([D, 512 * np_max], F32, tag="pp",
                                   name=f"{_rp}qp{k}")[:, 0:QW]
                    nc.tensor.matmul(qp, rh_k, qe, start=True, stop=True)
                    u1k = U1[:, k * QW:(k + 1) * QW]
                    _sign("dve" if k % 2 == 0 else "act", u1k, qp, nc)

            def stage_b(si, l):
                # projection + sign for one (slot, layer): 8 chunk matmuls,
                # 8 sign instructions (engine per _chunk_engine)
                b = _slot_order[si]
                pad_c = pads_c[b]
                splits = slot_splits[b]
                npieces = len(splits)
                e = st[si]["e"][:].rearrange(
                    "p (o c) -> p o c", o=L)[:, l, :][:, 0:pad_c]
                if l == 0:
                    st[si]["U2"] = jobp.tile([D, L * CH * pad_cmax], FP8,
                                             tag="U2", name=f"{_rp}U2{si}")
                U2 = st[si]["U2"][:, l * CH * pad_cmax:(l + 1) * CH * pad_cmax]
                for k in range(CH):
                    rh_k = rh[:, k * 128:(k + 1) * 128]
                    pp = ps_p.tile([D, 512 * np_max], F32, tag="pp",
                                   name=f"{_rp}pp{si}_{l}_{k}")
                    for c0, c1, p0 in splits:
                        nc.tensor.matmul(pp[:, p0:p0 + c1 - c0], rh_k,
                                         e[:, c0:c1], start=True, stop=True)
                    eng = _chunk_engine(l, k)
                    w = splits[0][1] - splits[0][0]
                    u2c = U2[:, k * pad_c:(k + 1) * pad_c]
                    if eng == "split":
                        # halves alternate DVE/ACT to fine-balance the slot
                        if npieces == 1:
                            h = (pad_c // 16) * 8
                            _sign("dve", u2c[:, 0:h], pp[:, 0:h], nc)
                            _sign("act", u2c[:, h:pad_c], pp[:, h:pad_c], nc)
                        else:
                            for pi, (c0, c1, p0) in enumerate(splits):
                                _sign("dve" if pi % 2 == 0 else "act",
                                      u2c[:, c0:c1], pp[:, p0:p0 + c1 - c0], nc)
                    elif npieces == 1:
                        _sign(eng, u2c, pp[:, 0:pad_c], nc)
                    elif npieces * w == pad_c:
                        ppv = pp[:].rearrange("p (n c) -> p n c",
                                              c=512)[:, 0:npieces, 0:w]
                        u2v = u2c.rearrange("p (n c) -> p n c", c=w)
                        _sign(eng, u2v, ppv, nc)
                    else:
                        for c0, c1, p0 in splits:
                            _sign(eng, u2c[:, c0:c1],
                                  pp[:, p0:p0 + c1 - c0], nc)

            def stage_c(si, l):
                b = _slot_order[si]
                pad_c = pads_c[b]
                splits = slot_splits[b]
                U2 = st[si]["U2"][:, l * CH * pad_cmax:(l + 1) * CH * pad_cmax]
                # code dot via fp8 DoubleRow into a per-layer S tile drawn
                # from the shared rotating pool (DoubleRow requires dst
                # partition base 0)
                S = ps_p.tile([qpad, 512 * np_max], F32, tag="pp",
                              name=f"{_rp}S{si}_{l}")
                st[si][f"S{l}"] = S
                qcol = (b * L + l) * qpad
                for c0, c1, p0 in splits:
                    w = c1 - c0
                    for jj in range(CH // 2):
                        lw = U1[:, 2 * jj * QW:(2 * jj + 2) * QW] \
                            .rearrange("p (o c) -> p o c", o=2)[:, :, qcol:qcol + qpad]
                        rv = U2[:, 2 * jj * pad_c:(2 * jj + 2) * pad_c] \
                            .rearrange("p (o c) -> p o c", o=2)[:, :, c0:c1]
                        nc.tensor.matmul(
                            S[:, p0:p0 + w], lw, rv,
                            start=(jj == 0), stop=(jj == CH // 2 - 1),
                            perf_mode=mybir.MatmulPerfMode.DoubleRow,
                        )

            def stage_d(si, split_dma=False):
                # per-layer Sin (fp16 out); one output DMA per slot, or one
                # per layer for the last slot so the final DMA overlaps the
                # second sin
                b = _slot_order[si]
                pad_c = pads_c[b]
                splits = slot_splits[b]
                npieces = len(splits)
                sim = outp.tile([qpad, L * pad_cmax], F16, tag="sim",
                                name=f"{_rp}sim{si}")
                w = splits[0][1] - splits[0][0]
                for l_ in range(L):
                    S = st[si][f"S{l_}"]
                    mvc = sim[:, l_ * pad_cmax:l_ * pad_cmax + pad_c]
                    if npieces == 1:
                        nc.scalar.activation(mvc, S[:, 0:pad_c], Act.Sin,
                                             scale=PI / (2.0 * NB))
                    elif npieces * w == pad_c:
                        sv = S[:].rearrange("p (n c) -> p n c",
                                            c=512)[:, 0:npieces, 0:w]
                        mv = mvc.rearrange("p (n c) -> p n c", c=w)
                        nc.scalar.activation(mv, sv, Act.Sin,
                                             scale=PI / (2.0 * NB))
                    else:
                        for c0, c1, p0 in splits:
                            nc.scalar.activation(mvc[:, c0:c1],
                                                 S[:, p0:p0 + c1 - c0],
                                                 Act.Sin, scale=PI / (2.0 * NB))
                    if split_dma:
                        nc.sync.dma_start(out=OUT[b, :, l_, 0:pad_c], in_=mvc)
                if not split_dma:
                    ov = OUT[b, :, :, 0:pad_c]
                    sv2 = sim[:].rearrange("p (o c) -> p o c", o=L)[:, :, 0:pad_c]
                    nc.sync.dma_start(out=ov, in_=sv2)

            # ---- emission: query first, then slot-pipelined doc stream.
            # b(si+1) is emitted BEFORE c(si)/d(si): per-engine streams run
            # in emission order, so the PE projects slot si+1 while ACT/DVE
            # drain slot si's signs, and the ACT sin for slot si queues
            # behind slot si+1's signs instead of blocking them. ----
            query_proj()
            stage_b(0, 0)
            stage_b(0, 1)
            for si in range(BPC):
                if si + 2 < BPC:
                    stage_a(si + 2)
                if si + 1 < BPC:
                    stage_b(si + 1, 0)
                    stage_b(si + 1, 1)
                stage_c(si, 0)
                stage_c(si, 1)
                stage_d(si, split_dma=(si == BPC - 1))

    nc.compile()
    return nc


def _tf32(x):
    """Round-to-nearest-even to 11-bit mantissa (bit-matches fp32_to_fp32r)."""
    u = np.ascontiguousarray(x, np.float32).view(np.uint32).astype(np.uint64)
    u = (u + 0x07FF + ((u >> 12) & 1)) & 0xFFFFFFFFFFFFF000
    return (u & 0xFFFFFFFF).astype(np.uint32).view(np.float32)


def _stage_inputs(query_embed, doc_embed, query_tok, doc_tok, r):
    query_embed = np.ascontiguousarray(query_embed, dtype=np.float32)
    doc_embed = np.ascontiguousarray(doc_embed, dtype=np.float32)
    r = np.ascontiguousarray(r, dtype=np.float32)

    qmask = (np.asarray(query_tok) != 0)
    dmask = (np.asarray(doc_tok) != 0)

    # sort batches by active count; slot s takes ranks [s*CORES, (s+1)*CORES)
    # spread across the 8 cores, so per-slot padding is tight and identical
    # on every core (SPMD requires one shape per slot)
    counts = dmask.sum(axis=1).astype(int)
    order = np.argsort(counts, kind="stable")
    assign = np.empty((CORES, BPC), dtype=int)   # assign[c, b] = batch id
    for s in range(BPC):
        for c in range(CORES):
            assign[c, s] = order[s * CORES + c]
    pads_c = tuple(
        min(BDOC, max(64, int(-(-int(counts[assign[:, s]].max()) // 16) * 16)))
        for s in range(BPC)
    )
    pad_cmax = max(pads_c)

    qe_m = query_embed * qmask[None, :, :, None].astype(np.float32)
    qidxs = [np.flatnonzero(qmask[g]) for g in range(BAT)]
    qpad = min(A, max(16, int(-(-max(len(q) for q in qidxs) // 8) * 8)))
    # r is pre-scaled and pre-rounded to tf32 (hw f32r read is then exact)
    rt = _tf32(np.ascontiguousarray(r.T * SCALE))

    idxs = [np.flatnonzero(dmask[g]) for g in range(BAT)]
    in_maps = []
    for c in range(CORES):
        # embeddings staged pre-transposed [D, tokens]; queries compacted
        # to their active rows (masks are per-batch, shared by both layers);
        # embeddings pre-rounded to tf32 so the hw f32r read is exact
        qe_c = np.zeros((D, BPC * L * qpad), dtype=np.float32)
        de_c = np.zeros((BPC, D, L, pad_cmax), dtype=np.float32)
        for b in range(BPC):
            g = assign[c, b]
            qi = qidxs[g]
            for li in range(L):
                col = (b * L + li) * qpad
                qe_c[:, col:col + len(qi)] = qe_m[li, g, qi].T
            idx = idxs[g]
            de_c[b, :, :, :len(idx)] = doc_embed[:, g, idx].transpose(2, 0, 1)
        in_maps.append({"qe": _tf32(qe_c), "de": _tf32(de_c), "rt": rt})

    return in_maps, assign, idxs, pads_c, qidxs, qpad


def kernel(query_embed, doc_embed, query_tok, doc_tok, r):
    in_maps, assign, idxs, pads_c, qidxs, qpad = _stage_inputs(
        query_embed, doc_embed, query_tok, doc_tok, r)

    key = (pads_c, qpad)
    if key not in _BUILD_CACHE:
        _BUILD_CACHE[key] = _build(pads_c, qpad)
    nc = _BUILD_CACHE[key]

    res = run_bass_kernel_spmd(nc, in_maps, core_ids=list(range(CORES)))

    out = np.zeros((BAT, L, A, BDOC), dtype=np.float32)
    for c in range(CORES):
        o_c = res.results[c]["out"]  # [BPC, qpad, L, pad_cmax] fp16 sim
        for b in range(BPC):
            g = assign[c, b]
            idx = idxs[g]
            qi = qidxs[g]
            for li in range(L):
                out[g, li][np.ix_(qi, idx)] = o_c[b, :len(qi), li, :len(idx)]
    return out
